# revision 43
# baseline (speedup 1.0000x reference)
"""Multi-head attention (B=2, T=2048, D=1024, H=16) on 8 TRN2 NeuronCores.

Sharding: 2D (batch x head-group). Core c handles batch b = c // 4 and head
group hg = c % 4 (4 heads = 256 channels of the projected dim). Each core:
  1. Projects its batch's q/k/v against its 256-row weight slices -> QT/KT
     in [j, t] layout and V in [t, j] layout (bf16, fp32 PSUM accumulation).
     V is stored augmented with a ones column per head: [V_h | 1].
     Order Q, V, K so attention never stalls waiting for V.
     Biases fold into the PSUM->SBUF copies as per-partition
     tensor_scalar adds (no PE bias matmuls).
  2. Per head pair, per 512-wide q tile: S.T = K_h @ Q_h.T (transposed
     scores), U = exp(S.T * scale) (no max subtraction: |S*scale| <= ~16,
     exp fits fp32 easily), then [O.T ; denom] += [V_h | 1].T @ U -- the
     softmax denominator rides the PV matmul for free as output row 64.
     The PV matmuls trail the score/exp stage by one k tile so the PE
     never waits on ScalarE (keeps the HAM clock at 2.4 GHz).
  3. Raw [O.T ; denom] is staged to SBUF; per-block reciprocals use the
     fast approx DVE op straight from PSUM; normalization + the output
     projection for q tile qt-1 are woven into the middle of qt's blocks
     as PE filler. The reciprocal broadcast runs as an f32r matmul and
     the normalization muls read it directly from PSUM.
  4. out_partial.T = woT_chunk.T @ O_norm.T -> [1024, 2048] bf16.
Host sums the 4 head-group partials per batch, transposes, adds bo.

PSUM discipline: exactly one accumulation group per PSUM bank (hardware
start=True clears has_written bits bank-wide). Engine ops only start at
partition offsets {0, 32, 64, 96}; partition shifts (head m=1 belongs at
rows 64-127 of the stage-E operand but results sit at rows 0-64) use
small SBUF->SBUF DMAs.

All shapes are hardcoded for this problem. kernel() takes the full inputs
and returns the full [2, 2048, 1024] fp32 output.
"""

import numpy as np
import ml_dtypes

import concourse.bass as bass
import concourse.bacc as bacc
import concourse.mybir as mybir
import concourse.tile as tile
from concourse.bass_utils import run_bass_kernel_spmd

B, T, D, H, Hd = 2, 2048, 1024, 16, 64
HPC = 4          # heads per core
W = HPC * Hd     # 256 projected channels per core
SCALE = Hd ** -0.5
N_CORES = 8

BF16 = mybir.dt.bfloat16
F32 = mybir.dt.float32
F32R = mybir.dt.float32r
bf16 = ml_dtypes.bfloat16


def build_nc():
    nc = bacc.Bacc("TRN2", target_bir_lowering=False, debug=False)

    xq = nc.dram_tensor("xq", [D, T], BF16, kind="ExternalInput").ap()
    xk = nc.dram_tensor("xk", [D, T], BF16, kind="ExternalInput").ap()
    xv = nc.dram_tensor("xv", [D, T], BF16, kind="ExternalInput").ap()
    # weights host-preswizzled to [128, chunk, cols] DMA-contiguous layout
    wq = nc.dram_tensor("wq", [128, 8 * W], BF16, kind="ExternalInput").ap()
    wk = nc.dram_tensor("wk", [128, 8 * W], BF16, kind="ExternalInput").ap()
    wv = nc.dram_tensor("wv", [128, 8 * W], BF16, kind="ExternalInput").ap()
    wo = nc.dram_tensor("wo", [128, 2 * D], BF16, kind="ExternalInput").ap()
    # biases as [128, 2] per-partition columns (b_col[p, jt] = b[jt*128+p])
    bq = nc.dram_tensor("bq", [128, 2], F32, kind="ExternalInput").ap()
    bk = nc.dram_tensor("bk", [128, 2], F32, kind="ExternalInput").ap()
    bv = nc.dram_tensor("bv", [128, 2], F32, kind="ExternalInput").ap()
    ident = nc.dram_tensor("ident", [128, 128], BF16, kind="ExternalInput").ap()
    out = nc.dram_tensor("out", [D, T], BF16, kind="ExternalOutput").ap()

    Exp = mybir.ActivationFunctionType.Exp

    with tile.TileContext(nc) as tc:
        with (
            tc.tile_pool(name="persist", bufs=1) as persist,
            tc.tile_pool(name="xpool", bufs=8) as xpool,
            tc.tile_pool(name="upool", bufs=8) as upool,
            tc.tile_pool(name="rpool", bufs=2) as rpool,
            tc.tile_pool(name="opool", bufs=4) as opool,
        ):
            # ---- weights / biases ----
            # K's weights first, per 128-row chunk -- the first matmul waits
            # only on chunk 0 of wk + the first half of xk's chunk 0.
            wk_sb = persist.tile([128, 8, W], BF16, tag="wk")
            wkr = wk.rearrange("p (c j) -> p c j", j=W)
            for c in range(8):
                nc.sync.dma_start(out=wk_sb[:, c, :], in_=wkr[:, c, :])
            # V/Q weights issue from the (idle in phase A) scalar HWDGE queue
            # so their issuance doesn't serialize behind wk on sync. V before
            # Q to match the projection order.
            wv_sb = persist.tile([128, 8, W], BF16, tag="wv")
            wvr = wv.rearrange("p (c j) -> p c j", j=W)
            for c in range(8):
                nc.scalar.dma_start(out=wv_sb[:, c, :], in_=wvr[:, c, :])
            wq_sb = persist.tile([128, 8, W], BF16, tag="wq")
            wqr = wq.rearrange("p (c j) -> p c j", j=W)
            for c in range(8):
                nc.scalar.dma_start(out=wq_sb[:, c, :], in_=wqr[:, c, :])
            wo_sb = persist.tile([128, 2, D], BF16, tag="wo")
            nc.sync.dma_start(out=wo_sb, in_=wo.rearrange("p (c e) -> p c e", e=D))
            bk_sb = persist.tile([128, 2], F32, tag="bk")
            nc.sync.dma_start(out=bk_sb, in_=bk)
            bq_sb = persist.tile([128, 2], F32, tag="bq")
            nc.sync.dma_start(out=bq_sb, in_=bq)
            bv_sb = persist.tile([128, 2], F32, tag="bv")
            nc.sync.dma_start(out=bv_sb, in_=bv)
            ident_sb = persist.tile([128, 128], BF16, tag="ident")
            nc.sync.dma_start(out=ident_sb, in_=ident)

            # ---- constants ----
            # ones row at partition 64: cols 0:64 are the reciprocal
            # broadcast matmul's stationary; the full row feeds the Pool
            # engine's ones/denom divide
            bcast1 = persist.tile([65, 512], BF16, tag="bcast1")
            nc.vector.memset(bcast1[64:65, :], 1.0)

            # ---- persistent activations ----
            qt_sb = persist.tile([128, 2, T], BF16, tag="qt")   # QT [j, t]
            kt_sb = persist.tile([128, 2, T], BF16, tag="kt")   # KT [j, t]
            # V augmented with ones column per head: [k, kt, h, 0:64]=V, [..64]=1
            vaug_sb = persist.tile([128, 16, HPC, Hd + 1], BF16, tag="vaug")
            nc.vector.memset(vaug_sb[:, :, :, 64:65], 1.0)
            otn_sb = persist.tile([128, 2, T], BF16, tag="otn")  # normalized O.T
            vt_sb = persist.tile([128, 2, T], BF16, tag="vt")    # V.T [j, t]
            # raw [O.T ; denom] per block b2 = (pr*4+qt)*2 + m
            oraw_sb = persist.tile([65, 16, 512], F32, tag="oraw")

            # ================= Phase A: projections =================
            with tc.tile_pool(name="psA", bufs=8, space="PSUM") as psA:
                def qk_proj(x_dram, w_sb, b_sb, dst, split_first=False):
                    ps = [psA.tile([128, 512], F32, tag="proj", name=f"proj{i}")
                          for i in range(8)]
                    for c in range(8):
                        xc = xpool.tile([128, T], BF16, tag="x", name="xc")
                        src = x_dram[c * 128:(c + 1) * 128, :]
                        if split_first and c == 0:
                            # halve the first chunk so matmuls start sooner
                            nc.gpsimd.dma_start(out=xc[:, 0:1024],
                                                in_=src[:, 0:1024])
                            nc.gpsimd.dma_start(out=xc[:, 1024:2048],
                                                in_=src[:, 1024:2048])
                        else:
                            nc.gpsimd.dma_start(out=xc, in_=src)
                        for jt in range(2):
                            for tt in range(4):
                                nc.tensor.matmul(
                                    ps[jt * 4 + tt],
                                    lhsT=w_sb[:, c, jt * 128:(jt + 1) * 128],
                                    rhs=xc[:, tt * 512:(tt + 1) * 512],
                                    start=(c == 0), stop=(c == 7),
                                )
                    for jt in range(2):
                        for tt in range(4):
                            # bias folds into the PSUM->SBUF copy
                            nc.vector.tensor_scalar_add(
                                dst[:, jt, tt * 512:(tt + 1) * 512],
                                ps[jt * 4 + tt], b_sb[:, jt:jt + 1])

                def v_transpose(jt, tt):
                    # one [128,128] slice of V.T -> vaug's [t, j] layout
                    tp = psA.tile([128, 128], BF16, tag="proj", name="tp")
                    nc.tensor.transpose(
                        tp, vt_sb[:, jt, tt * 128:(tt + 1) * 128], ident_sb)
                    nc.vector.tensor_copy(
                        vaug_sb[:, tt, 2 * jt:2 * jt + 2, 0:64],
                        tp.rearrange("t (h d) -> t h d", h=2))

                # K, then V, then Q: the attention blocks gate on Q (its
                # x arrives last over DMA), so everything else finishes
                # first and the blocks start the moment Q lands. The jt=0
                # V transposes (heads 0/1, needed first by the pr=0 blocks)
                # overlap the xq DMA window; jt=1 follows Q so the blocks
                # start ~1.7us after Q lands.
                with nc.named_scope("projK"):
                    qk_proj(xk, wk_sb, bk_sb, kt_sb, split_first=True)
                with nc.named_scope("projV"):
                    qk_proj(xv, wv_sb, bv_sb, vt_sb)
                with nc.named_scope("transA"):
                    for tt in range(16):
                        v_transpose(0, tt)
                with nc.named_scope("projQ"):
                    qk_proj(xq, wq_sb, bq_sb, qt_sb)
                with nc.named_scope("transB"):
                    for tt in range(16):
                        v_transpose(1, tt)

            # ====== Phase B/D + fused normalization/output projection ======
            with tc.tile_pool(name="psB", bufs=1, space="PSUM") as psB:
                recips = {}

                def emit_recip(b2, half):
                    # reciprocals run on DVE in [1,256] fragments woven into
                    # the NEXT block's slots so nothing latency-critical ever
                    # queues behind a long reciprocal
                    def run():
                        if half == 0:
                            recips[b2] = rpool.tile([65, 512], BF16,
                                                    tag="rtb", bufs=8,
                                                    name="rtb")
                        csl = slice(half * 256, (half + 1) * 256)
                        with nc.allow_low_precision(
                                reason="1/denom bf16; ample for softmax"):
                            nc.vector.reciprocal(recips[b2][64:65, csl],
                                                 oraw_sb[64:65, b2, csl])
                    return run

                def recip_fillers(pr, qt, slots):
                    # reciprocals for the PREVIOUS block's two denominators
                    if pr is None:
                        return []
                    b2 = (pr * 4 + qt) * 2
                    return [(slots[0], emit_recip(b2, 0)),
                            (slots[1], emit_recip(b2, 1)),
                            (slots[2], emit_recip(b2 + 1, 0)),
                            (slots[3], emit_recip(b2 + 1, 1))]



                def attn_block(pr, qt, fillers=()):
                    fillers = dict(fillers)
                    qsl = slice(qt * 512, (qt + 1) * 512)
                    o_psA = psB.tile([65, 512], F32, tag="oA", bufs=1,
                                     name="o_psA")
                    o_psB = psB.tile([65, 512], F32, tag="oB", bufs=1,
                                     name="o_psB")
                    us = []
                    for kt in range(17):
                        if kt < 16:
                            s_big = psB.tile([128, 2, 512], F32, tag="s",
                                             bufs=2, name="s_big")
                            for m in range(2):
                                po = 64 * m
                                nc.tensor.matmul(
                                    s_big[:, m, :],
                                    lhsT=kt_sb[po:po + 64, pr,
                                               kt * 128:(kt + 1) * 128],
                                    rhs=qt_sb[po:po + 64, pr, qsl],
                                    start=True, stop=True,
                                )
                            u_big = upool.tile([128, 2, 512], BF16, tag="u",
                                               name="u_big")
                            nc.scalar.activation(u_big, s_big, Exp, scale=SCALE)
                            us.append(u_big)
                        if kt >= 1:
                            for m, o_ps in ((0, o_psA), (1, o_psB)):
                                h = 2 * pr + m
                                nc.tensor.matmul(
                                    o_ps,
                                    lhsT=vaug_sb[:, kt - 1, h, :],
                                    rhs=us[kt - 1][:, m, :],
                                    start=(kt == 1), stop=(kt == 16),
                                )
                        # weave prior-tile normalization / projection work
                        # into the loop so ScalarE never starves at block
                        # boundaries
                        if kt in fillers:
                            fillers.pop(kt)()
                    for fn in fillers.values():
                        fn()
                    # fast approx reciprocal of each denominator row straight
                    # from PSUM (18-bit accurate, ~5x faster than the exact
                    # op), then stage raw results to SBUF.
                    # stage both raw results (frees the PSUM banks for the
                    # next block's PV); reciprocals are deferred into the
                    # next block in [1,256] fragments via emit_recip
                    for m, o_ps in ((0, o_psA), (1, o_psB)):
                        b2 = (pr * 4 + qt) * 2 + m
                        nc.vector.tensor_copy(oraw_sb[:, b2, :], o_ps)

                def norm_pieces(qt):
                    # normalize O.T for q tile qt: 4 filler closures
                    qsl = slice(qt * 512, (qt + 1) * 512)

                    def piece(pr, m):
                        def run():
                            b2 = (pr * 4 + qt) * 2 + m
                            rb_ps = psB.tile([64, 512], F32, tag="aux",
                                             bufs=2, name="rb_ps")
                            nc.tensor.matmul(
                                rb_ps, lhsT=bcast1[64:65, 0:64],
                                rhs=recips[b2][64:65, :],
                                start=True, stop=True)
                            # normalization muls read the broadcast straight
                            # from PSUM -- no staging copy
                            if m == 0:
                                nc.vector.tensor_mul(
                                    otn_sb[0:64, pr, qsl],
                                    oraw_sb[0:64, b2, :], rb_ps)
                            else:
                                otnB = rpool.tile([64, 512], BF16, tag="otnB",
                                                  name="otnB")
                                nc.vector.tensor_mul(
                                    otnB, oraw_sb[0:64, b2, :], rb_ps)
                                nc.sync.dma_start(
                                    out=otn_sb[64:128, pr, qsl], in_=otnB)
                        return run
                    # late slots: (pr=1) reciprocals are issued at the
                    # immediately preceding block boundary and take ~7us on
                    # DVE -- don't let the bcast matmul head-of-line-block
                    # the PE queue waiting for them
                    return [(9, piece(0, 0)), (11, piece(0, 1)),
                            (13, piece(1, 0)), (15, piece(1, 1))]

                def proj_pieces(qt, tail=False):
                    # output projection for q tile qt: 8 filler closures
                    qsl = slice(qt * 512, (qt + 1) * 512)

                    def piece(et):
                        def run():
                            e_ps = psB.tile([128, 512], F32, tag="aux",
                                            bufs=2, name="e_ps")
                            for jc in range(2):
                                nc.tensor.matmul(
                                    e_ps,
                                    lhsT=wo_sb[:, jc, et * 128:(et + 1) * 128],
                                    rhs=otn_sb[:, jc, qsl],
                                    start=(jc == 0), stop=(jc == 1),
                                )
                            stg = opool.tile([128, 512], BF16, tag="ostg",
                                             name="stg")
                            if tail and et % 2 == 1:
                                # exp work is over; ScalarE shares the final
                                # casts so they don't serialize on DVE
                                nc.scalar.copy(stg, e_ps)
                            else:
                                nc.vector.tensor_copy(stg, e_ps)
                            nc.sync.dma_start(
                                out=out[et * 128:(et + 1) * 128, qsl], in_=stg)
                        return run
                    # compressed into early slots so the PSUM-freeing casts
                    # all drain well before the next block's scores need the
                    # "s" ring slots back
                    return [(et + 2, piece(et)) for et in range(8)]

                prev = None
                for qt in range(4):
                    with nc.named_scope(f"blk0q{qt}"):
                        fl = recip_fillers(*(prev or (None, 0)),
                                           slots=(1, 3, 5, 7))
                        attn_block(0, qt, fillers=fl
                                   + (norm_pieces(qt - 1) if qt >= 1 else []))
                    prev = (0, qt)
                    with nc.named_scope(f"blk1q{qt}"):
                        fl = recip_fillers(*prev, slots=(11, 13, 15, 16))
                        attn_block(1, qt, fillers=fl
                                   + (proj_pieces(qt - 1) if qt >= 1 else []))
                    prev = (1, qt)
                with nc.named_scope("tail"):
                    # norm for pr=0 first (its reciprocals are long done),
                    # overlapping the final reciprocal pieces on DVE
                    nf = dict(norm_pieces(3))
                    nf.pop(9)(); nf.pop(11)()
                    for _, f in recip_fillers(*prev, slots=(0, 1, 2, 3)):
                        f()
                    nf.pop(13)(); nf.pop(15)()
                    for _, f in proj_pieces(3, tail=True):
                        f()

    nc.finalize()
    return nc


_NC_CACHE = None


def _get_nc():
    global _NC_CACHE
    if _NC_CACHE is None:
        _NC_CACHE = build_nc()
    return _NC_CACHE


def _swz(wT):
    """[C*128, cols] -> DMA-contiguous [128, C*cols] (partition-major)."""
    C = wT.shape[0] // 128
    return np.ascontiguousarray(
        wT.reshape(C, 128, -1).swapaxes(0, 1).reshape(128, -1)).astype(bf16)


def _bcol(b):
    """[W] -> [128, 2] per-partition bias columns."""
    return np.ascontiguousarray(
        np.asarray(b).reshape(2, 128).T).astype(np.float32)


def make_in_maps(query, key, value, wq, bq, wk, bk, wv, bv, wo, bo):
    in_maps = []
    for c in range(N_CORES):
        b, hg = divmod(c, HPC)
        sl = slice(hg * W, (hg + 1) * W)
        in_maps.append({
            "xq": np.ascontiguousarray(np.asarray(query[b]).T).astype(bf16),
            "xk": np.ascontiguousarray(np.asarray(key[b]).T).astype(bf16),
            "xv": np.ascontiguousarray(np.asarray(value[b]).T).astype(bf16),
            "wq": _swz(np.asarray(wq)[sl].T),
            "wk": _swz(np.asarray(wk)[sl].T),
            "wv": _swz(np.asarray(wv)[sl].T),
            "wo": _swz(np.asarray(wo)[:, sl].T),
            "bq": _bcol(np.asarray(bq)[sl]),
            "bk": _bcol(np.asarray(bk)[sl]),
            "bv": _bcol(np.asarray(bv)[sl]),
            "ident": np.eye(128, dtype=np.float32).astype(bf16),
        })
    return in_maps


def combine_outputs(outs, bo):
    full = np.zeros((B, T, D), np.float32)
    for c in range(N_CORES):
        b = c // HPC
        full[b] += outs[c].astype(np.float32).T
    full += np.asarray(bo, np.float32)[None, None, :]
    return full


def kernel(query, key, value, wq, bq, wk, bk, wv, bv, wo, bo):
    nc = _get_nc()
    in_maps = make_in_maps(query, key, value, wq, bq, wk, bk, wv, bv, wo, bo)
    res = run_bass_kernel_spmd(nc, in_maps, list(range(N_CORES)))
    outs = [np.asarray(res.results[c]["out"]) for c in range(N_CORES)]
    return combine_outputs(outs, bo)


# revision 44
# speedup vs baseline: 1.0677x; 1.0677x over previous
"""Multi-head attention (B=2, T=2048, D=1024, H=16) on 8 TRN2 NeuronCores.

Sharding: 2D (batch x head-group). Core c handles batch b = c // 4 and head
group hg = c % 4 (4 heads = 256 channels of the projected dim). Each core:
  1. Projects its batch's q/k/v against its 256-row weight slices -> QT/KT
     in [j, t] layout and V in [t, j] layout (bf16, fp32 PSUM accumulation).
     V is stored augmented with a ones column per head: [V_h | 1].
     Order Q, V, K so attention never stalls waiting for V.
     Biases fold into the PSUM->SBUF copies as per-partition
     tensor_scalar adds (no PE bias matmuls).
  2. Per head pair, per 512-wide q tile: S.T = K_h @ Q_h.T (transposed
     scores), U = exp(S.T * scale) (no max subtraction: |S*scale| <= ~16,
     exp fits fp32 easily), then [O.T ; denom] += [V_h | 1].T @ U -- the
     softmax denominator rides the PV matmul for free as output row 64.
     The PV matmuls trail the score/exp stage by one k tile so the PE
     never waits on ScalarE (keeps the HAM clock at 2.4 GHz).
  3. Raw [O.T ; denom] is staged to SBUF; per-block reciprocals use the
     fast approx DVE op straight from PSUM; normalization + the output
     projection for q tile qt-1 are woven into the middle of qt's blocks
     as PE filler. The reciprocal broadcast runs as an f32r matmul and
     the normalization muls read it directly from PSUM.
  4. out_partial.T = woT_chunk.T @ O_norm.T -> [1024, 2048] bf16.
Host sums the 4 head-group partials per batch, transposes, adds bo.

PSUM discipline: exactly one accumulation group per PSUM bank (hardware
start=True clears has_written bits bank-wide). Engine ops only start at
partition offsets {0, 32, 64, 96}; partition shifts (head m=1 belongs at
rows 64-127 of the stage-E operand but results sit at rows 0-64) use
small SBUF->SBUF DMAs.

All shapes are hardcoded for this problem. kernel() takes the full inputs
and returns the full [2, 2048, 1024] fp32 output.
"""

import numpy as np
import ml_dtypes

import concourse.bass as bass
import concourse.bacc as bacc
import concourse.mybir as mybir
import concourse.tile as tile
from concourse.bass_utils import run_bass_kernel_spmd

B, T, D, H, Hd = 2, 2048, 1024, 16, 64
HPC = 4          # heads per core
W = HPC * Hd     # 256 projected channels per core
SCALE = Hd ** -0.5
N_CORES = 8

BF16 = mybir.dt.bfloat16
F32 = mybir.dt.float32
F32R = mybir.dt.float32r
bf16 = ml_dtypes.bfloat16


def build_nc():
    nc = bacc.Bacc("TRN2", target_bir_lowering=False, debug=False)

    xq = nc.dram_tensor("xq", [D, T], BF16, kind="ExternalInput").ap()
    xk = nc.dram_tensor("xk", [D, T], BF16, kind="ExternalInput").ap()
    xv = nc.dram_tensor("xv", [D, T], BF16, kind="ExternalInput").ap()
    # weights host-preswizzled to [128, chunk, cols] DMA-contiguous layout
    wq = nc.dram_tensor("wq", [128, 8 * W], BF16, kind="ExternalInput").ap()
    wk = nc.dram_tensor("wk", [128, 8 * W], BF16, kind="ExternalInput").ap()
    wv = nc.dram_tensor("wv", [128, 8 * W], BF16, kind="ExternalInput").ap()
    wo = nc.dram_tensor("wo", [128, 2 * D], BF16, kind="ExternalInput").ap()
    # biases as [128, 2] per-partition columns (b_col[p, jt] = b[jt*128+p])
    bq = nc.dram_tensor("bq", [128, 2], F32, kind="ExternalInput").ap()
    bk = nc.dram_tensor("bk", [128, 2], F32, kind="ExternalInput").ap()
    bv = nc.dram_tensor("bv", [128, 2], F32, kind="ExternalInput").ap()
    ident = nc.dram_tensor("ident", [128, 128], BF16, kind="ExternalInput").ap()
    out = nc.dram_tensor("out", [D, T], BF16, kind="ExternalOutput").ap()

    Exp = mybir.ActivationFunctionType.Exp

    with tile.TileContext(nc) as tc:
        with (
            tc.tile_pool(name="persist", bufs=1) as persist,
            tc.tile_pool(name="xpool", bufs=8) as xpool,
            tc.tile_pool(name="upool", bufs=8) as upool,
            tc.tile_pool(name="rpool", bufs=2) as rpool,
            tc.tile_pool(name="opool", bufs=4) as opool,
        ):
            # ---- weights / biases ----
            # K's weights first, per 128-row chunk -- the first matmul waits
            # only on chunk 0 of wk + the first half of xk's chunk 0.
            wk_sb = persist.tile([128, 8, W], BF16, tag="wk")
            wkr = wk.rearrange("p (c j) -> p c j", j=W)
            for c in range(8):
                nc.sync.dma_start(out=wk_sb[:, c, :], in_=wkr[:, c, :])
            # V/Q weights also on sync (the scalar HWDGE queue is reserved
            # for the odd x chunks so x streams on two queues). V before Q
            # to match the projection order.
            wv_sb = persist.tile([128, 8, W], BF16, tag="wv")
            wvr = wv.rearrange("p (c j) -> p c j", j=W)
            for c in range(8):
                nc.sync.dma_start(out=wv_sb[:, c, :], in_=wvr[:, c, :])
            wq_sb = persist.tile([128, 8, W], BF16, tag="wq")
            wqr = wq.rearrange("p (c j) -> p c j", j=W)
            for c in range(8):
                nc.sync.dma_start(out=wq_sb[:, c, :], in_=wqr[:, c, :])
            wo_sb = persist.tile([128, 2, D], BF16, tag="wo")
            nc.sync.dma_start(out=wo_sb, in_=wo.rearrange("p (c e) -> p c e", e=D))
            bk_sb = persist.tile([128, 2], F32, tag="bk")
            nc.sync.dma_start(out=bk_sb, in_=bk)
            bq_sb = persist.tile([128, 2], F32, tag="bq")
            nc.sync.dma_start(out=bq_sb, in_=bq)
            bv_sb = persist.tile([128, 2], F32, tag="bv")
            nc.sync.dma_start(out=bv_sb, in_=bv)
            ident_sb = persist.tile([128, 128], BF16, tag="ident")
            nc.sync.dma_start(out=ident_sb, in_=ident)

            # ---- constants ----
            # ones row at partition 64: cols 0:64 are the reciprocal
            # broadcast matmul's stationary; the full row feeds the Pool
            # engine's ones/denom divide
            bcast1 = persist.tile([65, 512], BF16, tag="bcast1")
            nc.vector.memset(bcast1[64:65, :], 1.0)

            # ---- persistent activations ----
            qt_sb = persist.tile([128, 2, T], BF16, tag="qt")   # QT [j, t]
            kt_sb = persist.tile([128, 2, T], BF16, tag="kt")   # KT [j, t]
            # V augmented with ones column per head: [k, kt, h, 0:64]=V, [..64]=1
            vaug_sb = persist.tile([128, 16, HPC, Hd + 1], BF16, tag="vaug")
            nc.vector.memset(vaug_sb[:, :, :, 64:65], 1.0)
            otn_sb = persist.tile([128, 2, T], BF16, tag="otn")  # normalized O.T
            vt_sb = persist.tile([128, 2, T], BF16, tag="vt")    # V.T [j, t]
            # raw [O.T ; denom] per block b2 = (pr*4+qt)*2 + m
            oraw_sb = persist.tile([65, 16, 512], F32, tag="oraw")

            # ================= Phase A: projections =================
            with tc.tile_pool(name="psA", bufs=8, space="PSUM") as psA:
                def qk_proj(x_dram, w_sb, b_sb, dst, split_first=False):
                    ps = [psA.tile([128, 512], F32, tag="proj", name=f"proj{i}")
                          for i in range(8)]
                    for c in range(8):
                        xc = xpool.tile([128, T], BF16, tag="x", name="xc")
                        src = x_dram[c * 128:(c + 1) * 128, :]
                        # two DMA queues keep two transfers in flight; the
                        # scalar HWDGE queue carries nothing else in phase A
                        eng = nc.gpsimd if c % 2 == 0 else nc.scalar
                        if split_first and c == 0:
                            # halve the first chunk so matmuls start sooner
                            nc.gpsimd.dma_start(out=xc[:, 0:1024],
                                                in_=src[:, 0:1024])
                            nc.scalar.dma_start(out=xc[:, 1024:2048],
                                                in_=src[:, 1024:2048])
                        else:
                            eng.dma_start(out=xc, in_=src)
                        for jt in range(2):
                            for tt in range(4):
                                nc.tensor.matmul(
                                    ps[jt * 4 + tt],
                                    lhsT=w_sb[:, c, jt * 128:(jt + 1) * 128],
                                    rhs=xc[:, tt * 512:(tt + 1) * 512],
                                    start=(c == 0), stop=(c == 7),
                                )
                    for jt in range(2):
                        for tt in range(4):
                            # bias folds into the PSUM->SBUF copy
                            nc.vector.tensor_scalar_add(
                                dst[:, jt, tt * 512:(tt + 1) * 512],
                                ps[jt * 4 + tt], b_sb[:, jt:jt + 1])

                def v_transpose(jt, tt):
                    # one [128,128] slice of V.T -> vaug's [t, j] layout
                    tp = psA.tile([128, 128], BF16, tag="proj", name="tp")
                    nc.tensor.transpose(
                        tp, vt_sb[:, jt, tt * 128:(tt + 1) * 128], ident_sb)
                    nc.vector.tensor_copy(
                        vaug_sb[:, tt, 2 * jt:2 * jt + 2, 0:64],
                        tp.rearrange("t (h d) -> t h d", h=2))

                # K, then V, then Q: the attention blocks gate on Q (its
                # x arrives last over DMA), so everything else finishes
                # first and the blocks start the moment Q lands. The jt=0
                # V transposes (heads 0/1, needed first by the pr=0 blocks)
                # overlap the xq DMA window; jt=1 follows Q so the blocks
                # start ~1.7us after Q lands.
                with nc.named_scope("projK"):
                    qk_proj(xk, wk_sb, bk_sb, kt_sb, split_first=True)
                with nc.named_scope("projV"):
                    qk_proj(xv, wv_sb, bv_sb, vt_sb)
                with nc.named_scope("transA"):
                    for tt in range(16):
                        v_transpose(0, tt)
                with nc.named_scope("projQ"):
                    qk_proj(xq, wq_sb, bq_sb, qt_sb)
                with nc.named_scope("transB"):
                    for tt in range(16):
                        v_transpose(1, tt)

            # ====== Phase B/D + fused normalization/output projection ======
            with tc.tile_pool(name="psB", bufs=1, space="PSUM") as psB:
                recips = {}

                def emit_recip(b2, half):
                    # reciprocals run on DVE in [1,256] fragments woven into
                    # the NEXT block's slots so nothing latency-critical ever
                    # queues behind a long reciprocal
                    def run():
                        if half == 0:
                            recips[b2] = rpool.tile([65, 512], BF16,
                                                    tag="rtb", bufs=8,
                                                    name="rtb")
                        csl = slice(half * 256, (half + 1) * 256)
                        with nc.allow_low_precision(
                                reason="1/denom bf16; ample for softmax"):
                            nc.vector.reciprocal(recips[b2][64:65, csl],
                                                 oraw_sb[64:65, b2, csl])
                    return run

                def recip_fillers(pr, qt, slots):
                    # reciprocals for the PREVIOUS block's two denominators
                    if pr is None:
                        return []
                    b2 = (pr * 4 + qt) * 2
                    return [(slots[0], emit_recip(b2, 0)),
                            (slots[1], emit_recip(b2, 1)),
                            (slots[2], emit_recip(b2 + 1, 0)),
                            (slots[3], emit_recip(b2 + 1, 1))]



                def attn_block(pr, qt, fillers=()):
                    fillers = dict(fillers)
                    qsl = slice(qt * 512, (qt + 1) * 512)
                    o_psA = psB.tile([65, 512], F32, tag="oA", bufs=1,
                                     name="o_psA")
                    o_psB = psB.tile([65, 512], F32, tag="oB", bufs=1,
                                     name="o_psB")
                    us = []
                    for kt in range(17):
                        if kt < 16:
                            s_big = psB.tile([128, 2, 512], F32, tag="s",
                                             bufs=3, name="s_big")
                            for m in range(2):
                                po = 64 * m
                                nc.tensor.matmul(
                                    s_big[:, m, :],
                                    lhsT=kt_sb[po:po + 64, pr,
                                               kt * 128:(kt + 1) * 128],
                                    rhs=qt_sb[po:po + 64, pr, qsl],
                                    start=True, stop=True,
                                )
                            u_big = upool.tile([128, 2, 512], BF16, tag="u",
                                               name="u_big")
                            nc.scalar.activation(u_big, s_big, Exp, scale=SCALE)
                            us.append(u_big)
                        if kt >= 1:
                            for m, o_ps in ((0, o_psA), (1, o_psB)):
                                h = 2 * pr + m
                                nc.tensor.matmul(
                                    o_ps,
                                    lhsT=vaug_sb[:, kt - 1, h, :],
                                    rhs=us[kt - 1][:, m, :],
                                    start=(kt == 1), stop=(kt == 16),
                                )
                        # weave prior-tile normalization / projection work
                        # into the loop so ScalarE never starves at block
                        # boundaries
                        if kt in fillers:
                            fillers.pop(kt)()
                    for fn in fillers.values():
                        fn()
                    # fast approx reciprocal of each denominator row straight
                    # from PSUM (18-bit accurate, ~5x faster than the exact
                    # op), then stage raw results to SBUF.
                    # stage both raw results (frees the PSUM banks for the
                    # next block's PV); reciprocals are deferred into the
                    # next block in [1,256] fragments via emit_recip
                    for m, o_ps in ((0, o_psA), (1, o_psB)):
                        b2 = (pr * 4 + qt) * 2 + m
                        nc.vector.tensor_copy(oraw_sb[:, b2, :], o_ps)

                def norm_pieces(qt):
                    # normalize O.T for q tile qt: 4 filler closures
                    qsl = slice(qt * 512, (qt + 1) * 512)

                    def piece(pr, m):
                        def run():
                            b2 = (pr * 4 + qt) * 2 + m
                            rb_ps = psB.tile([64, 512], F32, tag="s", bufs=3,
                                             name="rb_ps")
                            nc.tensor.matmul(
                                rb_ps, lhsT=bcast1[64:65, 0:64],
                                rhs=recips[b2][64:65, :],
                                start=True, stop=True)
                            # normalization muls read the broadcast straight
                            # from PSUM -- no staging copy
                            if m == 0:
                                nc.vector.tensor_mul(
                                    otn_sb[0:64, pr, qsl],
                                    oraw_sb[0:64, b2, :], rb_ps)
                            else:
                                otnB = rpool.tile([64, 512], BF16, tag="otnB",
                                                  name="otnB")
                                nc.vector.tensor_mul(
                                    otnB, oraw_sb[0:64, b2, :], rb_ps)
                                nc.sync.dma_start(
                                    out=otn_sb[64:128, pr, qsl], in_=otnB)
                        return run
                    # late slots: (pr=1) reciprocals are issued at the
                    # immediately preceding block boundary and take ~7us on
                    # DVE -- don't let the bcast matmul head-of-line-block
                    # the PE queue waiting for them
                    return [(9, piece(0, 0)), (11, piece(0, 1)),
                            (13, piece(1, 0)), (15, piece(1, 1))]

                def proj_pieces(qt, tail=False):
                    # output projection for q tile qt: 8 filler closures
                    qsl = slice(qt * 512, (qt + 1) * 512)

                    def piece(et):
                        def run():
                            e_ps = psB.tile([128, 512], F32, tag="s", bufs=3,
                                            name="e_ps")
                            for jc in range(2):
                                nc.tensor.matmul(
                                    e_ps,
                                    lhsT=wo_sb[:, jc, et * 128:(et + 1) * 128],
                                    rhs=otn_sb[:, jc, qsl],
                                    start=(jc == 0), stop=(jc == 1),
                                )
                            stg = opool.tile([128, 512], BF16, tag="ostg",
                                             name="stg")
                            if tail and et % 2 == 1:
                                # exp work is over; ScalarE shares the final
                                # casts so they don't serialize on DVE
                                nc.scalar.copy(stg, e_ps)
                            else:
                                nc.vector.tensor_copy(stg, e_ps)
                            nc.sync.dma_start(
                                out=out[et * 128:(et + 1) * 128, qsl], in_=stg)
                        return run
                    # compressed into early slots so the PSUM-freeing casts
                    # all drain well before the next block's scores need the
                    # "s" ring slots back
                    return [(et + 2, piece(et)) for et in range(8)]

                prev = None
                for qt in range(4):
                    with nc.named_scope(f"blk0q{qt}"):
                        fl = recip_fillers(*(prev or (None, 0)),
                                           slots=(1, 3, 5, 7))
                        attn_block(0, qt, fillers=fl
                                   + (norm_pieces(qt - 1) if qt >= 1 else []))
                    prev = (0, qt)
                    with nc.named_scope(f"blk1q{qt}"):
                        fl = recip_fillers(*prev, slots=(11, 13, 15, 16))
                        attn_block(1, qt, fillers=fl
                                   + (proj_pieces(qt - 1) if qt >= 1 else []))
                    prev = (1, qt)
                with nc.named_scope("tail"):
                    # norm for pr=0 first (its reciprocals are long done),
                    # overlapping the final reciprocal pieces on DVE
                    nf = dict(norm_pieces(3))
                    nf.pop(9)(); nf.pop(11)()
                    for _, f in recip_fillers(*prev, slots=(0, 1, 2, 3)):
                        f()
                    nf.pop(13)(); nf.pop(15)()
                    for _, f in proj_pieces(3, tail=True):
                        f()

    nc.finalize()
    return nc


_NC_CACHE = None


def _get_nc():
    global _NC_CACHE
    if _NC_CACHE is None:
        _NC_CACHE = build_nc()
    return _NC_CACHE


def _swz(wT):
    """[C*128, cols] -> DMA-contiguous [128, C*cols] (partition-major)."""
    C = wT.shape[0] // 128
    return np.ascontiguousarray(
        wT.reshape(C, 128, -1).swapaxes(0, 1).reshape(128, -1)).astype(bf16)


def _bcol(b):
    """[W] -> [128, 2] per-partition bias columns."""
    return np.ascontiguousarray(
        np.asarray(b).reshape(2, 128).T).astype(np.float32)


def make_in_maps(query, key, value, wq, bq, wk, bk, wv, bv, wo, bo):
    in_maps = []
    for c in range(N_CORES):
        b, hg = divmod(c, HPC)
        sl = slice(hg * W, (hg + 1) * W)
        in_maps.append({
            "xq": np.ascontiguousarray(np.asarray(query[b]).T).astype(bf16),
            "xk": np.ascontiguousarray(np.asarray(key[b]).T).astype(bf16),
            "xv": np.ascontiguousarray(np.asarray(value[b]).T).astype(bf16),
            "wq": _swz(np.asarray(wq)[sl].T),
            "wk": _swz(np.asarray(wk)[sl].T),
            "wv": _swz(np.asarray(wv)[sl].T),
            "wo": _swz(np.asarray(wo)[:, sl].T),
            "bq": _bcol(np.asarray(bq)[sl]),
            "bk": _bcol(np.asarray(bk)[sl]),
            "bv": _bcol(np.asarray(bv)[sl]),
            "ident": np.eye(128, dtype=np.float32).astype(bf16),
        })
    return in_maps


def combine_outputs(outs, bo):
    full = np.zeros((B, T, D), np.float32)
    for c in range(N_CORES):
        b = c // HPC
        full[b] += outs[c].astype(np.float32).T
    full += np.asarray(bo, np.float32)[None, None, :]
    return full


def kernel(query, key, value, wq, bq, wk, bk, wv, bv, wo, bo):
    nc = _get_nc()
    in_maps = make_in_maps(query, key, value, wq, bq, wk, bk, wv, bv, wo, bo)
    res = run_bass_kernel_spmd(nc, in_maps, list(range(N_CORES)))
    outs = [np.asarray(res.results[c]["out"]) for c in range(N_CORES)]
    return combine_outputs(outs, bo)


# revision 45
# speedup vs baseline: 1.1224x; 1.0512x over previous
"""Multi-head attention (B=2, T=2048, D=1024, H=16) on 8 TRN2 NeuronCores.

Sharding: 2D (batch x head-group). Core c handles batch b = c // 4 and head
group hg = c % 4 (4 heads = 256 channels of the projected dim). Each core:
  1. Projects its batch's q/k/v against its 256-row weight slices -> QT/KT
     in [j, t] layout and V in [t, j] layout (bf16, fp32 PSUM accumulation).
     V is stored augmented with a ones column per head: [V_h | 1].
     Order Q, V, K so attention never stalls waiting for V.
     Biases fold into the PSUM->SBUF copies as per-partition
     tensor_scalar adds (no PE bias matmuls).
  2. Per head pair, per 512-wide q tile: S.T = K_h @ Q_h.T (transposed
     scores), U = exp(S.T * scale) (no max subtraction: |S*scale| <= ~16,
     exp fits fp32 easily), then [O.T ; denom] += [V_h | 1].T @ U -- the
     softmax denominator rides the PV matmul for free as output row 64.
     The PV matmuls trail the score/exp stage by one k tile so the PE
     never waits on ScalarE (keeps the HAM clock at 2.4 GHz).
  3. Raw [O.T ; denom] is staged to SBUF; per-block reciprocals use the
     fast approx DVE op straight from PSUM; normalization + the output
     projection for q tile qt-1 are woven into the middle of qt's blocks
     as PE filler. The reciprocal broadcast runs as an f32r matmul and
     the normalization muls read it directly from PSUM.
  4. out_partial.T = woT_chunk.T @ O_norm.T -> [1024, 2048] bf16.
Host sums the 4 head-group partials per batch, transposes, adds bo.

PSUM discipline: exactly one accumulation group per PSUM bank (hardware
start=True clears has_written bits bank-wide). Engine ops only start at
partition offsets {0, 32, 64, 96}; partition shifts (head m=1 belongs at
rows 64-127 of the stage-E operand but results sit at rows 0-64) use
small SBUF->SBUF DMAs.

All shapes are hardcoded for this problem. kernel() takes the full inputs
and returns the full [2, 2048, 1024] fp32 output.
"""

import numpy as np
import ml_dtypes

import concourse.bass as bass
import concourse.bacc as bacc
import concourse.mybir as mybir
import concourse.tile as tile
from concourse.bass_utils import run_bass_kernel_spmd

B, T, D, H, Hd = 2, 2048, 1024, 16, 64
HPC = 4          # heads per core
W = HPC * Hd     # 256 projected channels per core
SCALE = Hd ** -0.5
N_CORES = 8

BF16 = mybir.dt.bfloat16
F32 = mybir.dt.float32
F32R = mybir.dt.float32r
bf16 = ml_dtypes.bfloat16


def build_nc():
    nc = bacc.Bacc("TRN2", target_bir_lowering=False, debug=False)

    xq = nc.dram_tensor("xq", [D, T], BF16, kind="ExternalInput").ap()
    xk = nc.dram_tensor("xk", [D, T], BF16, kind="ExternalInput").ap()
    xv = nc.dram_tensor("xv", [D, T], BF16, kind="ExternalInput").ap()
    # weights host-preswizzled to [128, chunk, cols] DMA-contiguous layout
    wq = nc.dram_tensor("wq", [128, 8 * W], BF16, kind="ExternalInput").ap()
    wk = nc.dram_tensor("wk", [128, 8 * W], BF16, kind="ExternalInput").ap()
    wv = nc.dram_tensor("wv", [128, 8 * W], BF16, kind="ExternalInput").ap()
    wo = nc.dram_tensor("wo", [128, 2 * D], BF16, kind="ExternalInput").ap()
    # biases as [128, 2] per-partition columns (b_col[p, jt] = b[jt*128+p])
    bq = nc.dram_tensor("bq", [128, 2], F32, kind="ExternalInput").ap()
    bk = nc.dram_tensor("bk", [128, 2], F32, kind="ExternalInput").ap()
    bv = nc.dram_tensor("bv", [128, 2], F32, kind="ExternalInput").ap()
    ident = nc.dram_tensor("ident", [128, 128], BF16, kind="ExternalInput").ap()
    out = nc.dram_tensor("out", [D, T], BF16, kind="ExternalOutput").ap()

    Exp = mybir.ActivationFunctionType.Exp

    with tile.TileContext(nc) as tc:
        with (
            tc.tile_pool(name="persist", bufs=1) as persist,
            tc.tile_pool(name="xpool", bufs=8) as xpool,
            tc.tile_pool(name="upool", bufs=8) as upool,
            tc.tile_pool(name="rpool", bufs=2) as rpool,
            tc.tile_pool(name="opool", bufs=4) as opool,
        ):
            # ---- weights / biases ----
            # K's weights first, per 128-row chunk -- the first matmul waits
            # only on chunk 0 of wk + the first half of xk's chunk 0.
            wk_sb = persist.tile([128, 8, W], BF16, tag="wk")
            wkr = wk.rearrange("p (c j) -> p c j", j=W)
            for c in range(8):
                nc.sync.dma_start(out=wk_sb[:, c, :], in_=wkr[:, c, :])
            # V/Q weights issue from the (idle in phase A) scalar HWDGE
            # queue so their issuance doesn't serialize behind wk on sync.
            # V before Q to match the projection order.
            wv_sb = persist.tile([128, 8, W], BF16, tag="wv")
            wvr = wv.rearrange("p (c j) -> p c j", j=W)
            for c in range(8):
                nc.scalar.dma_start(out=wv_sb[:, c, :], in_=wvr[:, c, :])
            wq_sb = persist.tile([128, 8, W], BF16, tag="wq")
            wqr = wq.rearrange("p (c j) -> p c j", j=W)
            for c in range(8):
                nc.scalar.dma_start(out=wq_sb[:, c, :], in_=wqr[:, c, :])
            wo_sb = persist.tile([128, 2, D], BF16, tag="wo")
            nc.sync.dma_start(out=wo_sb, in_=wo.rearrange("p (c e) -> p c e", e=D))
            bk_sb = persist.tile([128, 2], F32, tag="bk")
            nc.sync.dma_start(out=bk_sb, in_=bk)
            bq_sb = persist.tile([128, 2], F32, tag="bq")
            nc.sync.dma_start(out=bq_sb, in_=bq)
            bv_sb = persist.tile([128, 2], F32, tag="bv")
            nc.sync.dma_start(out=bv_sb, in_=bv)
            ident_sb = persist.tile([128, 128], BF16, tag="ident")
            nc.sync.dma_start(out=ident_sb, in_=ident)

            # ---- constants ----
            # ones row at partition 64: cols 0:64 are the reciprocal
            # broadcast matmul's stationary; the full row feeds the Pool
            # engine's ones/denom divide
            bcast1 = persist.tile([65, 512], BF16, tag="bcast1")
            nc.vector.memset(bcast1[64:65, :], 1.0)

            # ---- persistent activations ----
            qt_sb = persist.tile([128, 2, T], BF16, tag="qt")   # QT [j, t]
            kt_sb = persist.tile([128, 2, T], BF16, tag="kt")   # KT [j, t]
            # V augmented with ones column per head: [k, kt, h, 0:64]=V, [..64]=1
            vaug_sb = persist.tile([128, 16, HPC, Hd + 1], BF16, tag="vaug")
            nc.vector.memset(vaug_sb[:, :, :, 64:65], 1.0)
            otn_sb = persist.tile([128, 2, T], BF16, tag="otn")  # normalized O.T
            vt_sb = persist.tile([128, 2, T], BF16, tag="vt")    # V.T [j, t]
            # raw [O.T ; denom] per block b2 = (pr*4+qt)*2 + m
            oraw_sb = persist.tile([65, 16, 512], F32, tag="oraw")

            # ================= Phase A: projections =================
            with tc.tile_pool(name="psA", bufs=8, space="PSUM") as psA:
                def qk_proj(x_dram, w_sb, b_sb, dst, split_first=False):
                    ps = [psA.tile([128, 512], F32, tag="proj", name=f"proj{i}")
                          for i in range(8)]
                    for c in range(8):
                        xc = xpool.tile([128, T], BF16, tag="x", name="xc")
                        src = x_dram[c * 128:(c + 1) * 128, :]
                        if split_first and c == 0:
                            # halve the first chunk so matmuls start sooner
                            nc.gpsimd.dma_start(out=xc[:, 0:1024],
                                                in_=src[:, 0:1024])
                            nc.gpsimd.dma_start(out=xc[:, 1024:2048],
                                                in_=src[:, 1024:2048])
                        else:
                            nc.gpsimd.dma_start(out=xc, in_=src)
                        for jt in range(2):
                            for tt in range(4):
                                nc.tensor.matmul(
                                    ps[jt * 4 + tt],
                                    lhsT=w_sb[:, c, jt * 128:(jt + 1) * 128],
                                    rhs=xc[:, tt * 512:(tt + 1) * 512],
                                    start=(c == 0), stop=(c == 7),
                                )
                    for jt in range(2):
                        for tt in range(4):
                            # bias folds into the PSUM->SBUF copy
                            nc.vector.tensor_scalar_add(
                                dst[:, jt, tt * 512:(tt + 1) * 512],
                                ps[jt * 4 + tt], b_sb[:, jt:jt + 1])

                def v_transpose(jt, tt):
                    # one [128,128] slice of V.T -> vaug's [t, j] layout
                    tp = psA.tile([128, 128], BF16, tag="proj", name="tp")
                    nc.tensor.transpose(
                        tp, vt_sb[:, jt, tt * 128:(tt + 1) * 128], ident_sb)
                    nc.vector.tensor_copy(
                        vaug_sb[:, tt, 2 * jt:2 * jt + 2, 0:64],
                        tp.rearrange("t (h d) -> t h d", h=2))

                # K, then V, then Q: the attention blocks gate on Q (its
                # x arrives last over DMA), so everything else finishes
                # first and the blocks start the moment Q lands. The jt=0
                # V transposes (heads 0/1, needed first by the pr=0 blocks)
                # overlap the xq DMA window; jt=1 follows Q so the blocks
                # start ~1.7us after Q lands.
                with nc.named_scope("projK"):
                    qk_proj(xk, wk_sb, bk_sb, kt_sb, split_first=True)
                with nc.named_scope("projV"):
                    qk_proj(xv, wv_sb, bv_sb, vt_sb)
                with nc.named_scope("transA"):
                    for tt in range(16):
                        v_transpose(0, tt)
                with nc.named_scope("projQ"):
                    qk_proj(xq, wq_sb, bq_sb, qt_sb)
                with nc.named_scope("transB"):
                    for tt in range(16):
                        v_transpose(1, tt)

            # ====== Phase B/D + fused normalization/output projection ======
            with tc.tile_pool(name="psB", bufs=1, space="PSUM") as psB:
                recips = {}

                def emit_recip(b2, half):
                    # reciprocals run on DVE in [1,256] fragments woven into
                    # the NEXT block's slots so nothing latency-critical ever
                    # queues behind a long reciprocal
                    def run():
                        if half == 0:
                            recips[b2] = rpool.tile([65, 512], BF16,
                                                    tag="rtb", bufs=8,
                                                    name="rtb")
                        csl = slice(half * 256, (half + 1) * 256)
                        with nc.allow_low_precision(
                                reason="1/denom bf16; ample for softmax"):
                            nc.vector.reciprocal(recips[b2][64:65, csl],
                                                 oraw_sb[64:65, b2, csl])
                    return run

                def recip_fillers(pr, qt, slots):
                    # reciprocals for the PREVIOUS block's two denominators
                    if pr is None:
                        return []
                    b2 = (pr * 4 + qt) * 2
                    return [(slots[0], emit_recip(b2, 0)),
                            (slots[1], emit_recip(b2, 1)),
                            (slots[2], emit_recip(b2 + 1, 0)),
                            (slots[3], emit_recip(b2 + 1, 1))]



                def attn_block(pr, qt, fillers=()):
                    fillers = dict(fillers)
                    qsl = slice(qt * 512, (qt + 1) * 512)
                    o_psA = psB.tile([65, 512], F32, tag="oA", bufs=1,
                                     name="o_psA")
                    o_psB = psB.tile([65, 512], F32, tag="oB", bufs=1,
                                     name="o_psB")
                    us = []
                    for kt in range(17):
                        if kt < 16:
                            s_big = psB.tile([128, 2, 512], F32, tag="s",
                                             bufs=3, name="s_big")
                            for m in range(2):
                                po = 64 * m
                                nc.tensor.matmul(
                                    s_big[:, m, :],
                                    lhsT=kt_sb[po:po + 64, pr,
                                               kt * 128:(kt + 1) * 128],
                                    rhs=qt_sb[po:po + 64, pr, qsl],
                                    start=True, stop=True,
                                )
                            u_big = upool.tile([128, 2, 512], BF16, tag="u",
                                               name="u_big")
                            nc.scalar.activation(u_big, s_big, Exp, scale=SCALE)
                            us.append(u_big)
                        if kt >= 1:
                            for m, o_ps in ((0, o_psA), (1, o_psB)):
                                h = 2 * pr + m
                                nc.tensor.matmul(
                                    o_ps,
                                    lhsT=vaug_sb[:, kt - 1, h, :],
                                    rhs=us[kt - 1][:, m, :],
                                    start=(kt == 1), stop=(kt == 16),
                                )
                        # weave prior-tile normalization / projection work
                        # into the loop so ScalarE never starves at block
                        # boundaries
                        if kt in fillers:
                            fillers.pop(kt)()
                    for fn in fillers.values():
                        fn()
                    # fast approx reciprocal of each denominator row straight
                    # from PSUM (18-bit accurate, ~5x faster than the exact
                    # op), then stage raw results to SBUF.
                    # stage both raw results (frees the PSUM banks for the
                    # next block's PV); reciprocals are deferred into the
                    # next block in [1,256] fragments via emit_recip
                    for m, o_ps in ((0, o_psA), (1, o_psB)):
                        b2 = (pr * 4 + qt) * 2 + m
                        nc.vector.tensor_copy(oraw_sb[:, b2, :], o_ps)

                def norm_pieces(qt):
                    # normalize O.T for q tile qt: 4 filler closures
                    qsl = slice(qt * 512, (qt + 1) * 512)

                    def piece(pr, m):
                        def run():
                            b2 = (pr * 4 + qt) * 2 + m
                            rb_ps = psB.tile([64, 512], F32, tag="s", bufs=3,
                                             name="rb_ps")
                            nc.tensor.matmul(
                                rb_ps, lhsT=bcast1[64:65, 0:64],
                                rhs=recips[b2][64:65, :],
                                start=True, stop=True)
                            # normalization muls read the broadcast straight
                            # from PSUM -- no staging copy
                            if m == 0:
                                nc.vector.tensor_mul(
                                    otn_sb[0:64, pr, qsl],
                                    oraw_sb[0:64, b2, :], rb_ps)
                            else:
                                otnB = rpool.tile([64, 512], BF16, tag="otnB",
                                                  name="otnB")
                                nc.vector.tensor_mul(
                                    otnB, oraw_sb[0:64, b2, :], rb_ps)
                                nc.sync.dma_start(
                                    out=otn_sb[64:128, pr, qsl], in_=otnB)
                        return run
                    # late slots: (pr=1) reciprocals are issued at the
                    # immediately preceding block boundary and take ~7us on
                    # DVE -- don't let the bcast matmul head-of-line-block
                    # the PE queue waiting for them
                    return [(9, piece(0, 0)), (11, piece(0, 1)),
                            (13, piece(1, 0)), (15, piece(1, 1))]

                def proj_pieces(qt, tail=False):
                    # output projection for q tile qt: 8 filler closures
                    qsl = slice(qt * 512, (qt + 1) * 512)

                    def piece(et):
                        def run():
                            e_ps = psB.tile([128, 512], F32, tag="s", bufs=3,
                                            name="e_ps")
                            for jc in range(2):
                                nc.tensor.matmul(
                                    e_ps,
                                    lhsT=wo_sb[:, jc, et * 128:(et + 1) * 128],
                                    rhs=otn_sb[:, jc, qsl],
                                    start=(jc == 0), stop=(jc == 1),
                                )
                            stg = opool.tile([128, 512], BF16, tag="ostg",
                                             name="stg")
                            if (tail and et % 2 == 1) or et in (3, 6):
                                # proj blocks are PE-bound with ~3us of
                                # ScalarE slack; sharing casts with ScalarE
                                # keeps the DVE queue clear of PSUM-freeing
                                # work (and in the tail exp is over entirely)
                                nc.scalar.copy(stg, e_ps)
                            else:
                                nc.vector.tensor_copy(stg, e_ps)
                            nc.sync.dma_start(
                                out=out[et * 128:(et + 1) * 128, qsl], in_=stg)
                        return run
                    # compressed into early slots so the PSUM-freeing casts
                    # all drain well before the next block's scores need the
                    # "s" ring slots back
                    return [(et + 2, piece(et)) for et in range(8)]

                prev = None
                for qt in range(4):
                    with nc.named_scope(f"blk0q{qt}"):
                        fl = recip_fillers(*(prev or (None, 0)),
                                           slots=(1, 3, 5, 7))
                        attn_block(0, qt, fillers=fl
                                   + (norm_pieces(qt - 1) if qt >= 1 else []))
                    prev = (0, qt)
                    with nc.named_scope(f"blk1q{qt}"):
                        fl = recip_fillers(*prev, slots=(11, 13, 15, 16))
                        attn_block(1, qt, fillers=fl
                                   + (proj_pieces(qt - 1) if qt >= 1 else []))
                    prev = (1, qt)
                with nc.named_scope("tail"):
                    # norm for pr=0 first (its reciprocals are long done),
                    # overlapping the final reciprocal pieces on DVE
                    nf = dict(norm_pieces(3))
                    nf.pop(9)(); nf.pop(11)()
                    for _, f in recip_fillers(*prev, slots=(0, 1, 2, 3)):
                        f()
                    nf.pop(13)(); nf.pop(15)()
                    for _, f in proj_pieces(3, tail=True):
                        f()

    nc.finalize()
    return nc


_NC_CACHE = None


def _get_nc():
    global _NC_CACHE
    if _NC_CACHE is None:
        _NC_CACHE = build_nc()
    return _NC_CACHE


def _swz(wT):
    """[C*128, cols] -> DMA-contiguous [128, C*cols] (partition-major)."""
    C = wT.shape[0] // 128
    return np.ascontiguousarray(
        wT.reshape(C, 128, -1).swapaxes(0, 1).reshape(128, -1)).astype(bf16)


def _bcol(b):
    """[W] -> [128, 2] per-partition bias columns."""
    return np.ascontiguousarray(
        np.asarray(b).reshape(2, 128).T).astype(np.float32)


def make_in_maps(query, key, value, wq, bq, wk, bk, wv, bv, wo, bo):
    in_maps = []
    for c in range(N_CORES):
        b, hg = divmod(c, HPC)
        sl = slice(hg * W, (hg + 1) * W)
        in_maps.append({
            "xq": np.ascontiguousarray(np.asarray(query[b]).T).astype(bf16),
            "xk": np.ascontiguousarray(np.asarray(key[b]).T).astype(bf16),
            "xv": np.ascontiguousarray(np.asarray(value[b]).T).astype(bf16),
            "wq": _swz(np.asarray(wq)[sl].T),
            "wk": _swz(np.asarray(wk)[sl].T),
            "wv": _swz(np.asarray(wv)[sl].T),
            "wo": _swz(np.asarray(wo)[:, sl].T),
            "bq": _bcol(np.asarray(bq)[sl]),
            "bk": _bcol(np.asarray(bk)[sl]),
            "bv": _bcol(np.asarray(bv)[sl]),
            "ident": np.eye(128, dtype=np.float32).astype(bf16),
        })
    return in_maps


def combine_outputs(outs, bo):
    full = np.zeros((B, T, D), np.float32)
    for c in range(N_CORES):
        b = c // HPC
        full[b] += outs[c].astype(np.float32).T
    full += np.asarray(bo, np.float32)[None, None, :]
    return full


def kernel(query, key, value, wq, bq, wk, bk, wv, bv, wo, bo):
    nc = _get_nc()
    in_maps = make_in_maps(query, key, value, wq, bq, wk, bk, wv, bv, wo, bo)
    res = run_bass_kernel_spmd(nc, in_maps, list(range(N_CORES)))
    outs = [np.asarray(res.results[c]["out"]) for c in range(N_CORES)]
    return combine_outputs(outs, bo)


# revision 48
# speedup vs baseline: 1.1246x; 1.0020x over previous
"""Multi-head attention (B=2, T=2048, D=1024, H=16) on 8 TRN2 NeuronCores.

Sharding: 2D (batch x head-group). Core c handles batch b = c // 4 and head
group hg = c % 4 (4 heads = 256 channels of the projected dim). Each core:
  1. Projects its batch's q/k/v against its 256-row weight slices -> QT/KT
     in [j, t] layout and V in [t, j] layout (bf16, fp32 PSUM accumulation).
     V is stored augmented with a ones column per head: [V_h | 1].
     Order Q, V, K so attention never stalls waiting for V.
     Biases fold into the PSUM->SBUF copies as per-partition
     tensor_scalar adds (no PE bias matmuls).
  2. Per head pair, per 512-wide q tile: S.T = K_h @ Q_h.T (transposed
     scores), U = exp(S.T * scale) (no max subtraction: |S*scale| <= ~16,
     exp fits fp32 easily), then [O.T ; denom] += [V_h | 1].T @ U -- the
     softmax denominator rides the PV matmul for free as output row 64.
     The PV matmuls trail the score/exp stage by one k tile so the PE
     never waits on ScalarE (keeps the HAM clock at 2.4 GHz).
  3. Raw [O.T ; denom] is staged to SBUF; per-block reciprocals use the
     fast approx DVE op straight from PSUM; normalization + the output
     projection for q tile qt-1 are woven into the middle of qt's blocks
     as PE filler. The reciprocal broadcast runs as an f32r matmul and
     the normalization muls read it directly from PSUM.
  4. out_partial.T = woT_chunk.T @ O_norm.T -> [1024, 2048] bf16.
Host sums the 4 head-group partials per batch, transposes, adds bo.

PSUM discipline: exactly one accumulation group per PSUM bank (hardware
start=True clears has_written bits bank-wide). Engine ops only start at
partition offsets {0, 32, 64, 96}; partition shifts (head m=1 belongs at
rows 64-127 of the stage-E operand but results sit at rows 0-64) use
small SBUF->SBUF DMAs.

All shapes are hardcoded for this problem. kernel() takes the full inputs
and returns the full [2, 2048, 1024] fp32 output.
"""

import numpy as np
import ml_dtypes

import concourse.bass as bass
import concourse.bacc as bacc
import concourse.mybir as mybir
import concourse.tile as tile
from concourse.bass_utils import run_bass_kernel_spmd

B, T, D, H, Hd = 2, 2048, 1024, 16, 64
HPC = 4          # heads per core
W = HPC * Hd     # 256 projected channels per core
SCALE = Hd ** -0.5
N_CORES = 8

BF16 = mybir.dt.bfloat16
F32 = mybir.dt.float32
F32R = mybir.dt.float32r
bf16 = ml_dtypes.bfloat16


def build_nc():
    nc = bacc.Bacc("TRN2", target_bir_lowering=False, debug=False)

    xq = nc.dram_tensor("xq", [D, T], BF16, kind="ExternalInput").ap()
    xk = nc.dram_tensor("xk", [D, T], BF16, kind="ExternalInput").ap()
    xv = nc.dram_tensor("xv", [D, T], BF16, kind="ExternalInput").ap()
    # weights host-preswizzled to [128, chunk, cols] DMA-contiguous layout
    wq = nc.dram_tensor("wq", [128, 8 * W], BF16, kind="ExternalInput").ap()
    wk = nc.dram_tensor("wk", [128, 8 * W], BF16, kind="ExternalInput").ap()
    wv = nc.dram_tensor("wv", [128, 8 * W], BF16, kind="ExternalInput").ap()
    wo = nc.dram_tensor("wo", [128, 2 * D], BF16, kind="ExternalInput").ap()
    # biases as [128, 2] per-partition columns (b_col[p, jt] = b[jt*128+p])
    bq = nc.dram_tensor("bq", [128, 2], F32, kind="ExternalInput").ap()
    bk = nc.dram_tensor("bk", [128, 2], F32, kind="ExternalInput").ap()
    bv = nc.dram_tensor("bv", [128, 2], F32, kind="ExternalInput").ap()
    ident = nc.dram_tensor("ident", [128, 128], BF16, kind="ExternalInput").ap()
    out = nc.dram_tensor("out", [D, T], BF16, kind="ExternalOutput").ap()

    Exp = mybir.ActivationFunctionType.Exp

    with tile.TileContext(nc) as tc:
        with (
            tc.tile_pool(name="persist", bufs=1) as persist,
            tc.tile_pool(name="xpool", bufs=8) as xpool,
            tc.tile_pool(name="upool", bufs=8) as upool,
            tc.tile_pool(name="rpool", bufs=2) as rpool,
            tc.tile_pool(name="opool", bufs=4) as opool,
        ):
            # ---- weights / biases ----
            # K's weights first, per 128-row chunk -- the first matmul waits
            # only on chunk 0 of wk + the first half of xk's chunk 0.
            wk_sb = persist.tile([128, 8, W], BF16, tag="wk")
            wkr = wk.rearrange("p (c j) -> p c j", j=W)
            for c in range(8):
                nc.sync.dma_start(out=wk_sb[:, c, :], in_=wkr[:, c, :])
            # V/Q weights issue from the (idle in phase A) scalar HWDGE
            # queue so their issuance doesn't serialize behind wk on sync.
            # V before Q to match the projection order.
            wv_sb = persist.tile([128, 8, W], BF16, tag="wv")
            wvr = wv.rearrange("p (c j) -> p c j", j=W)
            for c in range(8):
                nc.scalar.dma_start(out=wv_sb[:, c, :], in_=wvr[:, c, :])
            wq_sb = persist.tile([128, 8, W], BF16, tag="wq")
            wqr = wq.rearrange("p (c j) -> p c j", j=W)
            for c in range(8):
                nc.scalar.dma_start(out=wq_sb[:, c, :], in_=wqr[:, c, :])
            wo_sb = persist.tile([128, 2, D], BF16, tag="wo")
            nc.sync.dma_start(out=wo_sb, in_=wo.rearrange("p (c e) -> p c e", e=D))
            bk_sb = persist.tile([128, 2], F32, tag="bk")
            nc.sync.dma_start(out=bk_sb, in_=bk)
            bq_sb = persist.tile([128, 2], F32, tag="bq")
            nc.sync.dma_start(out=bq_sb, in_=bq)
            bv_sb = persist.tile([128, 2], F32, tag="bv")
            nc.sync.dma_start(out=bv_sb, in_=bv)
            ident_sb = persist.tile([128, 128], BF16, tag="ident")
            nc.sync.dma_start(out=ident_sb, in_=ident)

            # ---- constants ----
            # ones row at partition 64: cols 0:64 are the reciprocal
            # broadcast matmul's stationary; the full row feeds the Pool
            # engine's ones/denom divide
            bcast1 = persist.tile([65, 512], BF16, tag="bcast1")
            nc.vector.memset(bcast1[64:65, :], 1.0)

            # ---- persistent activations ----
            qt_sb = persist.tile([128, 2, T], BF16, tag="qt")   # QT [j, t]
            kt_sb = persist.tile([128, 2, T], BF16, tag="kt")   # KT [j, t]
            # V augmented with ones column per head: [k, kt, h, 0:64]=V, [..64]=1
            vaug_sb = persist.tile([128, 16, HPC, Hd + 1], BF16, tag="vaug")
            nc.vector.memset(vaug_sb[:, :, :, 64:65], 1.0)
            otn_sb = persist.tile([128, 2, T], BF16, tag="otn")  # normalized O.T
            vt_sb = persist.tile([128, 2, T], BF16, tag="vt")    # V.T [j, t]
            # raw [O.T ; denom] per block b2 = (pr*4+qt)*2 + m
            oraw_sb = persist.tile([65, 16, 512], F32, tag="oraw")

            # ================= Phase A: projections =================
            with tc.tile_pool(name="psA", bufs=8, space="PSUM") as psA:
                def qk_proj(x_dram, w_sb, b_sb, dst, split_first=False):
                    ps = [psA.tile([128, 512], F32, tag="proj", name=f"proj{i}")
                          for i in range(8)]
                    for c in range(8):
                        xc = xpool.tile([128, T], BF16, tag="x", name="xc")
                        src = x_dram[c * 128:(c + 1) * 128, :]
                        if split_first and c == 0:
                            # halve the first chunk so matmuls start sooner
                            nc.gpsimd.dma_start(out=xc[:, 0:1024],
                                                in_=src[:, 0:1024])
                            nc.gpsimd.dma_start(out=xc[:, 1024:2048],
                                                in_=src[:, 1024:2048])
                        else:
                            nc.gpsimd.dma_start(out=xc, in_=src)
                        for jt in range(2):
                            for tt in range(4):
                                nc.tensor.matmul(
                                    ps[jt * 4 + tt],
                                    lhsT=w_sb[:, c, jt * 128:(jt + 1) * 128],
                                    rhs=xc[:, tt * 512:(tt + 1) * 512],
                                    start=(c == 0), stop=(c == 7),
                                )
                    for jt in range(2):
                        for tt in range(4):
                            # bias folds into the PSUM->SBUF copy
                            nc.vector.tensor_scalar_add(
                                dst[:, jt, tt * 512:(tt + 1) * 512],
                                ps[jt * 4 + tt], b_sb[:, jt:jt + 1])

                def v_transpose(jt, tt):
                    # one [128,128] slice of V.T -> vaug's [t, j] layout
                    tp = psA.tile([128, 128], BF16, tag="proj", name="tp")
                    nc.tensor.transpose(
                        tp, vt_sb[:, jt, tt * 128:(tt + 1) * 128], ident_sb)
                    nc.vector.tensor_copy(
                        vaug_sb[:, tt, 2 * jt:2 * jt + 2, 0:64],
                        tp.rearrange("t (h d) -> t h d", h=2))

                # K, then V, then Q: the attention blocks gate on Q (its
                # x arrives last over DMA), so everything else finishes
                # first and the blocks start the moment Q lands. The jt=0
                # V transposes (heads 0/1, needed first by the pr=0 blocks)
                # overlap the xq DMA window; jt=1 follows Q so the blocks
                # start ~1.7us after Q lands.
                with nc.named_scope("projK"):
                    qk_proj(xk, wk_sb, bk_sb, kt_sb, split_first=True)
                with nc.named_scope("projV"):
                    qk_proj(xv, wv_sb, bv_sb, vt_sb)
                with nc.named_scope("transA"):
                    for tt in range(16):
                        v_transpose(0, tt)
                with nc.named_scope("projQ"):
                    qk_proj(xq, wq_sb, bq_sb, qt_sb)
                with nc.named_scope("transB"):
                    for tt in range(16):
                        v_transpose(1, tt)

            # ====== Phase B/D + fused normalization/output projection ======
            with tc.tile_pool(name="psB", bufs=1, space="PSUM") as psB:
                recips = {}

                def emit_recip(b2, half):
                    # reciprocals run on DVE in [1,256] fragments woven into
                    # the NEXT block's slots so nothing latency-critical ever
                    # queues behind a long reciprocal
                    def run():
                        if half == 0:
                            recips[b2] = rpool.tile([65, 512], BF16,
                                                    tag="rtb", bufs=8,
                                                    name="rtb")
                        csl = slice(half * 256, (half + 1) * 256)
                        with nc.allow_low_precision(
                                reason="1/denom bf16; ample for softmax"):
                            nc.vector.reciprocal(recips[b2][64:65, csl],
                                                 oraw_sb[64:65, b2, csl])
                    return run

                def recip_fillers(pr, qt, slots):
                    # reciprocals for the PREVIOUS block's two denominators
                    if pr is None:
                        return []
                    b2 = (pr * 4 + qt) * 2
                    return [(slots[0], emit_recip(b2, 0)),
                            (slots[1], emit_recip(b2, 1)),
                            (slots[2], emit_recip(b2 + 1, 0)),
                            (slots[3], emit_recip(b2 + 1, 1))]



                def attn_block(pr, qt, fillers=(), c0=0, cw=512):
                    # one flash block over q columns [qt*512+c0, +cw)
                    fillers = dict(fillers)
                    qsl = slice(qt * 512 + c0, qt * 512 + c0 + cw)
                    o_psA = psB.tile([65, cw], F32, tag="oA", bufs=1,
                                     name="o_psA")
                    o_psB = psB.tile([65, cw], F32, tag="oB", bufs=1,
                                     name="o_psB")
                    us = []
                    for kt in range(17):
                        if kt < 16:
                            s_big = psB.tile([128, 2, cw], F32, tag="s",
                                             bufs=3, name="s_big")
                            for m in range(2):
                                po = 64 * m
                                nc.tensor.matmul(
                                    s_big[:, m, :],
                                    lhsT=kt_sb[po:po + 64, pr,
                                               kt * 128:(kt + 1) * 128],
                                    rhs=qt_sb[po:po + 64, pr, qsl],
                                    start=True, stop=True,
                                )
                            u_big = upool.tile([128, 2, cw], BF16, tag="u",
                                               name="u_big")
                            nc.scalar.activation(u_big, s_big, Exp, scale=SCALE)
                            us.append(u_big)
                        if kt >= 1:
                            for m, o_ps in ((0, o_psA), (1, o_psB)):
                                h = 2 * pr + m
                                nc.tensor.matmul(
                                    o_ps,
                                    lhsT=vaug_sb[:, kt - 1, h, :],
                                    rhs=us[kt - 1][:, m, :],
                                    start=(kt == 1), stop=(kt == 16),
                                )
                        # weave prior-tile normalization / projection work
                        # into the loop so ScalarE never starves at block
                        # boundaries
                        if kt in fillers:
                            fillers.pop(kt)()
                    for fn in fillers.values():
                        fn()
                    # stage both raw results (frees the PSUM banks for the
                    # next block's PV); reciprocals are deferred into the
                    # next block in [1,256] fragments via emit_recip
                    for m, o_ps in ((0, o_psA), (1, o_psB)):
                        b2 = (pr * 4 + qt) * 2 + m
                        nc.vector.tensor_copy(
                            oraw_sb[:, b2, c0:c0 + cw], o_ps)

                def norm_piece(qt, pr, m, c0=0, cw=512):
                    def run():
                        qsl = slice(qt * 512 + c0, qt * 512 + c0 + cw)
                        b2 = (pr * 4 + qt) * 2 + m
                        rb_ps = psB.tile([64, cw], F32, tag="s", bufs=3,
                                         name="rb_ps")
                        nc.tensor.matmul(
                            rb_ps, lhsT=bcast1[64:65, 0:64],
                            rhs=recips[b2][64:65, c0:c0 + cw],
                            start=True, stop=True)
                        # normalization muls read the broadcast straight
                        # from PSUM -- no staging copy
                        if m == 0:
                            nc.vector.tensor_mul(
                                otn_sb[0:64, pr, qsl],
                                oraw_sb[0:64, b2, c0:c0 + cw], rb_ps)
                        else:
                            otnB = rpool.tile([64, cw], BF16, tag="otnB",
                                              name="otnB")
                            nc.vector.tensor_mul(
                                otnB, oraw_sb[0:64, b2, c0:c0 + cw], rb_ps)
                            nc.sync.dma_start(
                                out=otn_sb[64:128, pr, qsl], in_=otnB)
                    return run

                def norm_pieces(qt):
                    # normalize O.T for q tile qt: 4 filler closures.
                    # late slots: (pr=1) reciprocals are issued at the
                    # immediately preceding block boundary and take ~7us on
                    # DVE -- don't let the bcast matmul head-of-line-block
                    # the PE queue waiting for them
                    return [(9, norm_piece(qt, 0, 0)),
                            (11, norm_piece(qt, 0, 1)),
                            (13, norm_piece(qt, 1, 0)),
                            (15, norm_piece(qt, 1, 1))]

                def proj_piece(qt, et, c0=0, cw=512, cast_eng="v"):
                    def run():
                        qsl = slice(qt * 512 + c0, qt * 512 + c0 + cw)
                        e_ps = psB.tile([128, cw], F32, tag="s", bufs=3,
                                        name="e_ps")
                        for jc in range(2):
                            nc.tensor.matmul(
                                e_ps,
                                lhsT=wo_sb[:, jc, et * 128:(et + 1) * 128],
                                rhs=otn_sb[:, jc, qsl],
                                start=(jc == 0), stop=(jc == 1),
                            )
                        stg = opool.tile([128, cw], BF16, tag="ostg",
                                         name="stg")
                        if cast_eng == "s":
                            # ScalarE shares casts where it has slack so the
                            # DVE queue stays clear of PSUM-freeing work
                            nc.scalar.copy(stg, e_ps)
                        else:
                            nc.vector.tensor_copy(stg, e_ps)
                        nc.sync.dma_start(
                            out=out[et * 128:(et + 1) * 128, qsl], in_=stg)
                    return run

                def proj_pieces(qt):
                    # output projection for q tile qt: 8 filler closures,
                    # compressed into early slots so the PSUM-freeing casts
                    # all drain well before the next block's scores need the
                    # "s" ring slots back
                    return [(et + 2,
                             proj_piece(qt, et,
                                        cast_eng="s" if et in (3, 6) else "v"))
                            for et in range(8)]

                prev = None
                for qt in range(4):
                    with nc.named_scope(f"blk0q{qt}"):
                        fl = recip_fillers(*(prev or (None, 0)),
                                           slots=(1, 3, 5, 7))
                        attn_block(0, qt, fillers=fl
                                   + (norm_pieces(qt - 1) if qt >= 1 else []))
                    prev = (0, qt)
                    with nc.named_scope(f"blk1q{qt}"):
                        fl = recip_fillers(*prev, slots=(11, 13, 15, 16))
                        attn_block(1, qt, fillers=fl
                                   + (proj_pieces(qt - 1) if qt >= 1 else []))
                    prev = (1, qt)
                with nc.named_scope("tail"):
                    # norm for pr=0 first (its reciprocals are long done),
                    # overlapping the final reciprocal pieces on DVE
                    nf = dict(norm_pieces(3))
                    nf.pop(9)(); nf.pop(11)()
                    for _, f in recip_fillers(*prev, slots=(0, 1, 2, 3)):
                        f()
                    nf.pop(13)(); nf.pop(15)()
                    for et in range(8):
                        proj_piece(3, et,
                                   cast_eng="s" if et % 2 else "v")()

    nc.finalize()
    return nc


_NC_CACHE = None


def _get_nc():
    global _NC_CACHE
    if _NC_CACHE is None:
        _NC_CACHE = build_nc()
    return _NC_CACHE


def _swz(wT):
    """[C*128, cols] -> DMA-contiguous [128, C*cols] (partition-major)."""
    C = wT.shape[0] // 128
    return np.ascontiguousarray(
        wT.reshape(C, 128, -1).swapaxes(0, 1).reshape(128, -1)).astype(bf16)


def _bcol(b):
    """[W] -> [128, 2] per-partition bias columns."""
    return np.ascontiguousarray(
        np.asarray(b).reshape(2, 128).T).astype(np.float32)


def make_in_maps(query, key, value, wq, bq, wk, bk, wv, bv, wo, bo):
    in_maps = []
    for c in range(N_CORES):
        b, hg = divmod(c, HPC)
        sl = slice(hg * W, (hg + 1) * W)
        in_maps.append({
            "xq": np.ascontiguousarray(np.asarray(query[b]).T).astype(bf16),
            "xk": np.ascontiguousarray(np.asarray(key[b]).T).astype(bf16),
            "xv": np.ascontiguousarray(np.asarray(value[b]).T).astype(bf16),
            "wq": _swz(np.asarray(wq)[sl].T),
            "wk": _swz(np.asarray(wk)[sl].T),
            "wv": _swz(np.asarray(wv)[sl].T),
            "wo": _swz(np.asarray(wo)[:, sl].T),
            "bq": _bcol(np.asarray(bq)[sl]),
            "bk": _bcol(np.asarray(bk)[sl]),
            "bv": _bcol(np.asarray(bv)[sl]),
            "ident": np.eye(128, dtype=np.float32).astype(bf16),
        })
    return in_maps


def combine_outputs(outs, bo):
    full = np.zeros((B, T, D), np.float32)
    for c in range(N_CORES):
        b = c // HPC
        full[b] += outs[c].astype(np.float32).T
    full += np.asarray(bo, np.float32)[None, None, :]
    return full


def kernel(query, key, value, wq, bq, wk, bk, wv, bv, wo, bo):
    nc = _get_nc()
    in_maps = make_in_maps(query, key, value, wq, bq, wk, bk, wv, bv, wo, bo)
    res = run_bass_kernel_spmd(nc, in_maps, list(range(N_CORES)))
    outs = [np.asarray(res.results[c]["out"]) for c in range(N_CORES)]
    return combine_outputs(outs, bo)
